# revision 25
# baseline (speedup 1.0000x reference)
"""Trainium2 Bass kernel for ChebyshevActivation.

Math:
    scale = clip(input_scale, 0.1, 2.0)
    t = tanh(x * scale)                        # t in (-1, 1)
    out[b, o] = sum_w coeffs[o, w] * sum_i T_w(t[b, i])

Since |t| < 1 the reference's clip(+-100) is dead code.  We work in the
monomial basis: with power sums M_j[b] = sum_i t[b,i]^j (M_0 = IN_F
exactly) and G = coeffs @ C (C the Chebyshev->monomial matrix),
out = M @ G^T.

Op selection is driven by the cost model's DVE fast-mode table:
  - InstTensorScalarPtr with TWO tensor operands (scalar_tensor_tensor)
    runs 1x, but the scalar-immediate form (tensor_scalar, two-op variant
    with accum_out) runs 4x on fp16 -> a full-row moment extraction is
    ~533ns instead of ~2133ns.
  - tensor_tensor (mult) runs 2x on fp16 -> product streams at ~1067ns.
  - ACT activations (tanh/square) are 1x but a parallel engine, and their
    fused accum_out gives the stream's moment for ~187ns extra.
  - GPSIMD tensor_tensor (walrus-accepted here) adds a third elementwise
    engine at ~2.03ns/col for junk product streams.

Streams: t1=tanh(x), t2=t1^2 (ACT), t3=t2*t1 (DVE), t4=t2^2 (ACT);
junk products t5=t4*t1, t6=t3^2, t7=t4*t3, t8=t4^2 carry M5..M8.  Every
junk stream's columns can be split ACT(square-only)/GPS/DVE per CFG; each
ACT piece accumulates its own m-column, the GPS+DVE columns land in one
junk tile read by a single 4x tensor_scalar accum.  The host duplicates
the matching G rows so the final PE matmul re-merges all pieces.

Per-core layout: data-parallel over batch, 8 cores x 1024 rows,
8 row-tiles of [128, 2048] per core.
"""

import numpy as np

import concourse.bass as bass
import concourse.bacc as bacc
import concourse.mybir as mybir
import concourse.tile as tile
from concourse import masks
from concourse.bass_utils import run_bass_kernel_spmd

# This environment's walrus build rejects raw client-encoded ISA instructions
# ("ISA wrong length" for the 64-byte EVENT_SEMAPHORE_RANGE_CLEAR emitted by
# the TileContext exit barrier).  Replace the range-clear with per-semaphore
# EventSemaphore writes (update_mode=sem-wr-imm, value 0), which this walrus
# accepts, so re-executing the loaded NEFF still sees cleared semaphores.
def _sem_clear_via_events(self, sem_range):
    engines = list(self.bass.engines.values())
    inst = None
    for i, s in enumerate(sem_range):
        eng = engines[i % len(engines)]
        inst = mybir.InstEventSemaphore(
            name=self.bass.get_next_instruction_name(),
            ins=[], outs=[],
            sync_info=mybir.SyncInfo(
                on_wait=[],
                on_update=[mybir.SyncUpdate(
                    sync_type="semaphore", id=s,
                    update_mode="sem-wr-imm", update_value=0,
                )],
            ),
        )
        eng.add_instruction(inst)
    return inst


bass.BassGpSimd.sem_clear = _sem_clear_via_events

N_CORES = 8
BATCH = 8192
IN_F = 2048
OUT_F = 1024
DEG = 8
W = DEG + 1
ROWS_PER_CORE = BATCH // N_CORES  # 1024
P = 128
NTILES = ROWS_PER_CORE // P  # 8

F32 = mybir.dt.float32
F16 = mybir.dt.float16
MULT = mybir.AluOpType.mult
ADD = mybir.AluOpType.add
SQUARE = mybir.ActivationFunctionType.Square
TANH = mybir.ActivationFunctionType.Tanh

# Column splits per junk stream s: (act_cols, gps_cols); DVE takes the rest.
# act pieces are squares so only t6 (=t3^2) and t8 (=t4^2) can use ACT.
CFG = {
    "a2": IN_F,      # ACT cols of t2 square (value stream)
    "a4": IN_F,      # ACT cols of t4 square (value stream)
    "a6": 1024,      # ACT cols of t6 (M6)
    "a8": 0,         # ACT cols of t8 (M8)
    "g5": 2048,      # GPS cols of t5 (M5)
    "g7": 896,       # GPS cols of t7 (M7)
    "g8": 0,         # GPS cols of t8 (M8)
    "oc_act": 0,     # out-copy cols on ACT (rest DVE)
    "warm": True,    # split tile 0's x DMA + tanh into halves
    "tail_delay": 2,  # software-pipeline depth for the per-tile tail
    "mt_act": True,   # moment-transpose PSUM->SBUF copy on ACT
    "gt_act": True,   # gt DMA on the ACT HWDGE queue (SP queue free for x)
    "xin_bufs": 3,
    "val_bufs": 3,
    "junk_bufs": 3,
    "ostage_bufs": 3,
    "tail_fast": True,
}


def _cheb_monomial_matrix(deg=DEG):
    C = np.zeros((deg + 1, deg + 1), dtype=np.float64)
    C[0, 0] = 1.0
    if deg >= 1:
        C[1, 1] = 1.0
    for n in range(2, deg + 1):
        C[n, 1:] = 2.0 * C[n - 1, :-1]
        C[n, :] -= C[n - 2, :]
    return C


def _plan(cfg):
    """Ordered m-column list: (moment_k, tag). Single source of truth for
    both the kernel emission and the host GT row duplication."""
    cols = [(0, "M0"), (1, "M1")]
    if cfg["a2"] > 0:
        cols.append((2, "M2a"))
    if cfg["a2"] < IN_F:
        cols.append((2, "M2d"))
    if cfg["a4"] > 0:
        cols.append((4, "M4a"))
    if cfg["a4"] < IN_F:
        cols.append((4, "M4d"))
    cols.append((3, "M3"))
    for k, a_key, g_key in ((5, None, "g5"), (6, "a6", None),
                            (7, None, "g7"), (8, "a8", "g8")):
        a = cfg.get(a_key, 0) if a_key else 0
        g = cfg.get(g_key, 0) if g_key else 0
        if a > 0:
            cols.append((k, f"M{k}a"))
        if a + g < IN_F or g > 0:
            cols.append((k, f"M{k}d"))
    if cfg.get("warm"):
        cols.append((1, "W1"))
    if cfg.get("warm2"):
        cols.append((2, "W2"))
    return cols


def _build_nc(scale: float, cfg=CFG) -> bass.Bass:
    plan = _plan(cfg)
    K = len(plan)
    assert K <= 32
    idx = {tag: i for i, (_k, tag) in enumerate(plan)}

    nc = bacc.Bacc("TRN2")
    x = nc.dram_tensor("x", [ROWS_PER_CORE, IN_F], F32, kind="ExternalInput")
    gt = nc.dram_tensor("gt", [K, OUT_F], F32, kind="ExternalInput")
    out = nc.dram_tensor("out", [ROWS_PER_CORE, OUT_F], F32,
                         kind="ExternalOutput")

    with tile.TileContext(nc) as tc:
        with (
            tc.tile_pool(name="singles", bufs=1) as singles,
            tc.tile_pool(name="xin", bufs=cfg["xin_bufs"]) as xin,
            tc.tile_pool(name="vals", bufs=cfg["val_bufs"]) as vals,
            tc.tile_pool(name="junk", bufs=cfg["junk_bufs"]) as junkp,
            tc.tile_pool(name="janx", bufs=2) as janx,
            tc.tile_pool(name="mpool", bufs=4) as mpool,
            tc.tile_pool(name="mtsb", bufs=4) as mtsb,
            tc.tile_pool(name="ostage", bufs=cfg["ostage_bufs"]) as ostage,
            tc.tile_pool(name="pt", bufs=cfg.get("pt_bufs", 2),
                         space="PSUM") as pt,
            tc.tile_pool(name="pout", bufs=cfg.get("pout_bufs", 2),
                         space="PSUM") as pout,
        ):
            gt_sb = singles.tile([K, OUT_F], F32)
            ident = singles.tile([P, P], F32)
            if not cfg.get("gt_late"):
                # gt on the ACT HWDGE queue keeps the SP queue free for x(0)
                geng = nc.scalar if cfg.get("gt_act") else nc.sync
                geng.dma_start(out=gt_sb[:, :], in_=gt[:, :])
                masks.make_identity(nc, ident[:, :])

            def front(it):
                """Tile front: DMA, tanh, value squares, product streams.
                Returns the closure state for the deferred tail."""
                r0 = it * P
                x_t = xin.tile([P, IN_F], F32)
                chunked = cfg.get("warm") and it == 0
                H = IN_F // 2
                if chunked:
                    nc.sync.dma_start(out=x_t[:, 0:H], in_=x[r0:r0 + P, 0:H])
                    nc.sync.dma_start(out=x_t[:, H:IN_F],
                                      in_=x[r0:r0 + P, H:IN_F])
                else:
                    nc.sync.dma_start(out=x_t[:, :], in_=x[r0:r0 + P, :])
                if it == 0 and cfg.get("gt_late"):
                    # gt + identity after tile 0's x so tanh starts sooner
                    nc.sync.dma_start(out=gt_sb[:, :], in_=gt[:, :])
                    masks.make_identity(nc, ident[:, :])

                m_t = mpool.tile([P, K], F32)
                nc.gpsimd.memset(m_t[:, 0:1], float(IN_F))
                if cfg.get("warm") and not chunked:
                    w = idx["W1"]
                    nc.gpsimd.memset(m_t[:, w:w + 1], 0.0)
                if cfg.get("warm2") and not chunked:
                    w = idx["W2"]
                    nc.gpsimd.memset(m_t[:, w:w + 1], 0.0)

                def mcol(tag):
                    i = idx[tag]
                    return m_t[:, i:i + 1]

                t1 = vals.tile([P, IN_F], F16, tag="t1")
                t2 = vals.tile([P, IN_F], F16, tag="t2")
                t3 = vals.tile([P, IN_F], F16, tag="t3")
                t4 = vals.tile([P, IN_F], F16, tag="t4")

                # t1 = tanh(scale*x), accum -> M1 (warm: halves on tile 0)
                if chunked:
                    nc.scalar.activation(out=t1[:, 0:H], in_=x_t[:, 0:H],
                                         func=TANH, scale=scale,
                                         accum_out=mcol("M1"))
                    nc.scalar.activation(out=t1[:, H:IN_F], in_=x_t[:, H:IN_F],
                                         func=TANH, scale=scale,
                                         accum_out=mcol("W1"))
                else:
                    nc.scalar.activation(out=t1[:, :], in_=x_t[:, :],
                                         func=TANH, scale=scale,
                                         accum_out=mcol("M1"))

                def val_square(src, dst, a_cols, tag_a, tag_d):
                    """Value stream dst = src^2: ACT [0:a] + DVE [a:IN_F]."""
                    if a_cols > 0:
                        nc.scalar.activation(out=dst[:, 0:a_cols],
                                             in_=src[:, 0:a_cols], func=SQUARE,
                                             accum_out=mcol(tag_a))
                    if a_cols < IN_F:
                        nc.vector.tensor_tensor(out=dst[:, a_cols:IN_F],
                                                in0=src[:, a_cols:IN_F],
                                                in1=src[:, a_cols:IN_F],
                                                op=MULT)
                        nc.vector.tensor_scalar(out=dst[:, a_cols:IN_F],
                                                in0=dst[:, a_cols:IN_F],
                                                scalar1=1.0, scalar2=0.0,
                                                op0=MULT, op1=ADD,
                                                accum_out=mcol(tag_d))

                if chunked and cfg.get("warm2"):
                    # cascade tile 0 halves: sq2 + tt3 start one half sooner
                    nc.scalar.activation(out=t2[:, 0:H], in_=t1[:, 0:H],
                                         func=SQUARE, accum_out=mcol("M2a"))
                    nc.scalar.activation(out=t2[:, H:IN_F], in_=t1[:, H:IN_F],
                                         func=SQUARE, accum_out=mcol("W2"))
                    nc.vector.tensor_tensor(out=t3[:, 0:H], in0=t2[:, 0:H],
                                            in1=t1[:, 0:H], op=MULT)
                    nc.vector.tensor_tensor(out=t3[:, H:IN_F],
                                            in0=t2[:, H:IN_F],
                                            in1=t1[:, H:IN_F], op=MULT)
                else:
                    val_square(t1, t2, cfg["a2"], "M2a", "M2d")
                    nc.vector.tensor_tensor(out=t3[:, :], in0=t2[:, :],
                                            in1=t1[:, :], op=MULT)
                tsd = junkp.tile([P, IN_F], F16, tag="tsd")
                nc.vector.tensor_scalar(out=tsd[:, :], in0=t3[:, :],
                                        scalar1=1.0, scalar2=0.0,
                                        op0=MULT, op1=ADD,
                                        accum_out=mcol("M3"))

                val_square(t2, t4, cfg["a4"], "M4a", "M4d")

                deferred = []
                jd_of = {}

                def junk_moment(k, in0, in1, a_cols, g_cols, gps_ins=None):
                    """Junk product stream: ACT square [0:a] (own accum,
                    deferred to the tail since it may wait on DVE's t3),
                    GPS tt [a:a+g], DVE tt [a+g:IN_F]; one 4x ts accum over
                    [a:IN_F], deferred when fed by the slow GPSIMD engine so
                    the in-order DVE queue never stalls on it."""
                    if a_cols > 0:
                        def emit_sq(k=k, in0=in0, a_cols=a_cols):
                            ja = janx.tile([P, a_cols], F16, tag=f"ja{k}")
                            nc.scalar.activation(out=ja[:, :],
                                                 in_=in0[:, 0:a_cols],
                                                 func=SQUARE,
                                                 accum_out=mcol(f"M{k}a"))
                        deferred.append(emit_sq)
                    lo = a_cols
                    mid = min(IN_F, a_cols + g_cols)
                    if lo >= IN_F:
                        return
                    jd = junkp.tile([P, IN_F - lo], F16, tag=f"jd{k}")
                    jd_of[k] = jd
                    if mid > lo:
                        gi0, gi1 = gps_ins if gps_ins else (in0, in1)
                        nc.gpsimd.tensor_tensor(out=jd[:, 0:mid - lo],
                                                in0=gi0[:, lo:mid],
                                                in1=gi1[:, lo:mid], op=MULT)
                    if mid < IN_F:
                        nc.vector.tensor_tensor(out=jd[:, mid - lo:],
                                                in0=in0[:, mid:IN_F],
                                                in1=in1[:, mid:IN_F], op=MULT)

                    def emit_ts(jd=jd, k=k):
                        nc.vector.tensor_scalar(out=jd[:, :], in0=jd[:, :],
                                                scalar1=1.0, scalar2=0.0,
                                                op0=MULT, op1=ADD,
                                                accum_out=mcol(f"M{k}d"))
                    if mid > lo:
                        deferred.append(emit_ts)
                    else:
                        emit_ts()

                if it == NTILES - 1:
                    # taper GPSIMD on the final tile: its tail has no later
                    # front work to overlap the slow Pool engine with
                    g5 = cfg.get("g5_last", 0 if cfg.get("last_no_gps")
                                 else cfg["g5"])
                    g7 = cfg.get("g7_last", 0 if cfg.get("last_no_gps")
                                 else cfg["g7"])
                    g8 = 0 if cfg.get("last_no_gps") else cfg["g8"]
                else:
                    g5, g7, g8 = cfg["g5"], cfg["g7"], cfg["g8"]
                if cfg.get("gps_chain"):
                    # GPS streams avoid t4 (the longest ACT chain): t5 = t2*t3
                    # and t7 = jd5*t2 (Pool self-chain, valid if g7 <= g5).
                    assert g7 <= g5 or g5 == 0
                    junk_moment(5, t2, t3, 0, g5)
                    junk_moment(8, t4, t4, cfg["a8"], g8)
                    junk_moment(6, t3, t3, cfg["a6"], 0)
                    junk_moment(7, t4, t3, 0, g7,
                                gps_ins=(jd_of[5], t2) if g5 else None)
                else:
                    junk_moment(8, t4, t4, cfg["a8"], g8)
                    junk_moment(6, t3, t3, cfg["a6"], 0)
                    junk_moment(5, t4, t1, 0, g5)
                    junk_moment(7, t4, t3, 0, g7)
                return {"it": it, "r0": r0, "m_t": m_t, "deferred": deferred}

            def tail(st):
                """Tile tail, emitted one iteration later so its cross-engine
                waits overlap the next tile's front work."""
                it, r0, m_t = st["it"], st["r0"], st["m_t"]
                for emit in st["deferred"]:
                    emit()

                # Transpose moments [P, K] -> [K, P] PSUM, copy to SBUF
                mt_ps = pt.tile([K, P], F32)
                nc.tensor.transpose(mt_ps[:, :], m_t[:, :], ident[:, :])
                mt_sb = mtsb.tile([K, P], F32)
                if cfg.get("mt_act"):
                    nc.scalar.copy(mt_sb[:, :], mt_ps[:, :])
                else:
                    nc.vector.tensor_copy(mt_sb[:, :], mt_ps[:, :])

                # out[128, 1024] = MT.T @ GT  (contraction K).  f32r runs the
                # PE at 1 cycle/row instead of f32's 4 (same bits, TF32-like
                # precision -- fine at our tolerance).
                F32R = mybir.dt.float32r
                lhs_ap = mt_sb[:, :]
                o_ps = pout.tile([P, OUT_F], F32)
                for h in range(2):
                    rhs_ap = gt_sb[:, h * 512:(h + 1) * 512]
                    if cfg.get("f32r"):
                        nc.tensor.matmul(o_ps[:, h * 512:(h + 1) * 512],
                                         lhsT=lhs_ap.bitcast(F32R),
                                         rhs=rhs_ap.bitcast(F32R),
                                         start=True, stop=True)
                    else:
                        nc.tensor.matmul(o_ps[:, h * 512:(h + 1) * 512],
                                         lhsT=lhs_ap, rhs=rhs_ap,
                                         start=True, stop=True)
                o_sb = ostage.tile([P, OUT_F], F32)
                ca = cfg["oc_act"]
                if cfg.get("tail_fast") and it == NTILES - 1:
                    H2 = OUT_F // 2
                    nc.scalar.copy(o_sb[:, 0:H2], o_ps[:, 0:H2])
                    nc.vector.tensor_copy(o_sb[:, H2:OUT_F], o_ps[:, H2:OUT_F])
                    nc.sync.dma_start(out=out[r0:r0 + P, 0:H2],
                                      in_=o_sb[:, 0:H2])
                    nc.sync.dma_start(out=out[r0:r0 + P, H2:OUT_F],
                                      in_=o_sb[:, H2:OUT_F])
                    return
                if ca > 0:
                    nc.scalar.copy(o_sb[:, 0:ca], o_ps[:, 0:ca])
                if ca < OUT_F:
                    nc.vector.tensor_copy(o_sb[:, ca:OUT_F], o_ps[:, ca:OUT_F])
                nc.sync.dma_start(out=out[r0:r0 + P, :], in_=o_sb[:, :])

            pending = []
            D = cfg.get("tail_delay", 1)
            for it in range(NTILES):
                st = front(it)
                pending.append(st)
                if len(pending) > D:
                    tail(pending.pop(0))
            for st in pending:
                tail(st)

    nc.finalize()
    return nc


_NC_CACHE: dict[tuple, bass.Bass] = {}


def _host_gt(coeffs, cfg=CFG):
    C = _cheb_monomial_matrix()
    G = (coeffs.astype(np.float64) @ C).astype(np.float32)  # [OUT_F, W]
    rows = [k for k, _tag in _plan(cfg)]
    GT = np.ascontiguousarray(G.T[rows, :])  # [K, OUT_F]
    return GT


def _run(x, coeffs, input_scale, cfg=CFG, **spmd_kwargs):
    x = np.ascontiguousarray(np.asarray(x, dtype=np.float32))
    coeffs = np.asarray(coeffs, dtype=np.float32)
    scale = float(np.clip(np.asarray(input_scale, dtype=np.float32),
                          0.1, 2.0).reshape(-1)[0])

    GT = _host_gt(coeffs, cfg)

    key = (scale, str(cfg))
    nc = _NC_CACHE.get(key)
    if nc is None:
        nc = _build_nc(scale, cfg)
        _NC_CACHE[key] = nc

    in_maps = [
        {"x": np.ascontiguousarray(x[c * ROWS_PER_CORE:(c + 1) * ROWS_PER_CORE]),
         "gt": GT}
        for c in range(N_CORES)
    ]
    res = run_bass_kernel_spmd(nc, in_maps, core_ids=list(range(N_CORES)),
                               **spmd_kwargs)
    out = np.concatenate([res.results[c]["out"] for c in range(N_CORES)],
                         axis=0)
    return out.astype(np.float32), res


def kernel(x, coeffs, input_scale):
    out, _ = _run(x, coeffs, input_scale)
    return out


if __name__ == "__main__":
    rng = np.random.default_rng(0)
    x = rng.standard_normal((BATCH, IN_F), dtype=np.float32)
    coeffs = (rng.standard_normal((OUT_F, W)) * 0.1).astype(np.float32)
    s = np.ones((1,), np.float32)
    out = kernel(x=x, coeffs=coeffs, input_scale=s)
    print(out.shape, out.dtype)
